# revision 1
# baseline (speedup 1.0000x reference)
"""NT-Xent loss kernel for Trainium2, 8 NeuronCores.

Problem: B=4096 per view, D=128, temperature=0.1.
reps = concat([zjs, zis]) -> [8192, 128]; normalize rows; sim = normed @ normed.T;
loss = mean_i(-pos_i/T + logsumexp_{j!=i}(sim_ij/T)).

Strategy (fully static SPMD, no collectives):
  Each core c receives reps rotated by -1024*c rows, host-packed into the
  on-chip tile layout [p, t, d] as bf16 (the matmul operand precision anyway)
  so the input load is 2MB of fully-contiguous DMA, split across the two
  HWDGE queues (SP + ACT engines).  In the rotated frame the core's 1024 rows
  are rows 0..1023, the diagonal for row-chunk mi sits at columns
  128mi..128mi+127 of column-group 0, and the positive column for row
  128mi+p is 4096+128mi+p.  Per core:
    Phase A: row sums of squares (fp32 accumulation of exact bf16 products),
             rsqrt via exp(-0.5*ln(ss)) (single ACT table set), normalize to
             bf16 via one broadcast-AP multiply per 4-tile slice, xbar-DMA-
             transpose slices into HIT [128d, 8192rows] (bf16).
             diag_i = ||hi_i||^2 and pos_i come from row-major dots emitted
             early so they never gate the ScalarE pipeline.
    Phase B: per column group q (4) x row-chunk mi (8): 4 bf16 matmuls
             (N=512) -> PSUM [128, 2048]; ScalarE Exp(10x-10) in place with
             accum_out producing the partial row sums of exp.
  Per-row bf16 noise (~2e-3) is zero-mean; averaged over 8192 rows the final
  scalar keeps ~1e-5 relative accuracy.  Output per core: [128, 8] per-row
  losses; host sums in float64.
"""

import numpy as np

B = 4096
D = 128
TWO_B = 2 * B
P = 128
NTILE = TWO_B // P        # 64 row tiles
MI = 8                    # row-chunks per core (128 rows each -> 1024 rows)
GQ = 4                    # column groups of 2048
GW = TWO_B // GQ          # 2048 columns per group
TPG = GW // P             # 16 tiles per group
SUB = 4                   # tiles per fine-grained norm/transpose slice
NCORES = 8
ROWS_PER_CORE = TWO_B // NCORES  # 1024
INV_T = 10.0              # 1 / temperature
SHIFT = 10.0              # fixed logsumexp shift (sim/T <= 10)

_CACHE = {}


def build_nc():
    import concourse.bacc as bacc
    import concourse.bass as bass
    import concourse.mybir as mybir
    import concourse.tile as tile

    f32 = mybir.dt.float32
    bf16 = mybir.dt.bfloat16
    AX = mybir.AxisListType
    OP = mybir.AluOpType
    AF = mybir.ActivationFunctionType

    # Make the act-table chooser pick the one set that holds BOTH Ln and Exp
    # (otherwise it alternates exp_and_others <-> natural_log, ~1.3us per
    # reload).  Only the choice is restricted; the chosen set's real runtime
    # contents still cover every function we emit.
    from concourse import hw_specs

    _orig_tables = hw_specs.get_activation_tables

    def _patched_tables(arch):
        t = {k: set(v) for k, v in _orig_tables(arch).items()}
        for name, s in t.items():
            if name != "natural_log_exp_and_others":
                s.discard(AF.Exp)
                s.discard(AF.Ln)
        return t

    bacc.get_activation_tables = _patched_tables

    nc = bacc.Bacc(
        "TRN2",
        target_bir_lowering=False,
        debug=False,
        num_devices=NCORES,
    )
    # host-tiled: reps_h[p, t*128 + d] = bf16(reps_rot[t*128 + p, d])
    reps_h = nc.declare_dram_parameter("reps", [P, TWO_B], bf16, isOutput=False)
    out_h = nc.declare_dram_parameter("out", [P, MI], f32, isOutput=True)

    ident_dram = nc.inline_tensor(np.eye(P, dtype=np.float32), name="ident_const")

    with tile.TileContext(nc) as tc:
        with (
            tc.tile_pool(name="persist", bufs=1) as persist,
            tc.tile_pool(name="psum", bufs=2, space="PSUM") as psum,
            tc.tile_pool(name="scratch", bufs=2) as scratch,
        ):
            ident = persist.tile([P, P], f32)
            nc.gpsimd.dma_start(out=ident, in_=ident_dram[:, :])
            bias_shift = persist.tile([P, 1], f32)
            nc.vector.memset(bias_shift, -SHIFT)

            RAW = persist.tile([P, NTILE, P], bf16)
            SQ = persist.tile([P, NTILE, P], f32)
            HI = persist.tile([P, NTILE, P], bf16)
            HIT = persist.tile([P, TWO_B], bf16)
            SS = persist.tile([P, NTILE], f32)
            SSC = persist.tile([P, NTILE], f32)
            LNSS = persist.tile([P, NTILE], f32)
            SCL = persist.tile([P, NTILE], f32)
            SPART = persist.tile([P, MI, GQ], f32)
            DIAG = persist.tile([P, MI], f32)
            POS = persist.tile([P, MI], f32)

            # ---------------- Phase A: load + normalize + transpose ---------
            reps_t = reps_h[:, :].rearrange("p (t d) -> p t d", d=P)
            HIT3 = HIT.rearrange("d (t p) -> d t p", p=P)

            def scl_bcast(a, b):
                # SCL[:, a:b] broadcast along a trailing step-0 dim of width P
                s = SCL[:, a:b]
                return bass.AP(
                    tensor=s.tensor, offset=s.offset, ap=list(s.ap) + [[0, P]]
                )

            def slice_stats(x, y):
                nc.vector.tensor_mul(SQ[:, x:y, :], RAW[:, x:y, :], RAW[:, x:y, :])
                nc.vector.reduce_sum(out=SS[:, x:y], in_=SQ[:, x:y, :], axis=AX.X)
                # norm clamp: max(||r||, 1e-8) == sqrt(max(ss, 1e-16))
                nc.vector.tensor_scalar_max(out=SSC[:, x:y], in0=SS[:, x:y],
                                            scalar1=1e-16)
                # rsqrt(ss) = exp(-0.5*ln(ss))
                nc.scalar.activation(out=LNSS[:, x:y], in_=SSC[:, x:y], func=AF.Ln)
                nc.scalar.activation(out=SCL[:, x:y], in_=LNSS[:, x:y],
                                     func=AF.Exp, scale=-0.5)

            def slice_norm(x, y, eng):
                nc.vector.tensor_mul(HI[:, x:y, :], RAW[:, x:y, :], scl_bcast(x, y))
                eng.dma_start_transpose(out=HIT3[:, x:y, :], in_=HI[:, x:y, :])

            def phase_a_group0():
                # group 0 gates Phase B: 4-tile pieces on alternating load
                # queues, per-piece stats, transposes split over BOTH HWDGE
                # engines (ScalarE is idle this early)
                for s, (ld, tr) in enumerate(
                    [(nc.scalar, nc.sync), (nc.gpsimd, nc.scalar)] * 2
                ):
                    x, y = s * SUB, (s + 1) * SUB
                    ld.dma_start(out=RAW[:, x:y, :], in_=reps_t[:, x:y, :])
                    slice_stats(x, y)
                    slice_norm(x, y, tr)

            def phase_a_group(g):
                a, b = g * TPG, (g + 1) * TPG
                # loads only on the ACT-HWDGE and SWDGE queues: the sync
                # queue is reserved for the transposes so group 0's
                # transposes aren't stuck behind groups 1-3's loads
                splits = [(a, a + 10, nc.scalar), (a + 10, b, nc.gpsimd)]
                for x, y, eng in splits:
                    eng.dma_start(out=RAW[:, x:y, :], in_=reps_t[:, x:y, :])
                for x, y, _ in splits:
                    nc.vector.tensor_mul(
                        SQ[:, x:y, :], RAW[:, x:y, :], RAW[:, x:y, :]
                    )
                    nc.vector.reduce_sum(
                        out=SS[:, x:y], in_=SQ[:, x:y, :], axis=AX.X
                    )
                nc.vector.tensor_scalar_max(
                    out=SSC[:, a:b], in0=SS[:, a:b], scalar1=1e-16
                )
                nc.scalar.activation(out=LNSS[:, a:b], in_=SSC[:, a:b], func=AF.Ln)
                nc.scalar.activation(
                    out=SCL[:, a:b], in_=LNSS[:, a:b], func=AF.Exp, scale=-0.5
                )
                for x in range(a, b, SUB):
                    nc.vector.tensor_mul(
                        HI[:, x : x + SUB, :],
                        RAW[:, x : x + SUB, :],
                        scl_bcast(x, x + SUB),
                    )
                    nc.sync.dma_start_transpose(
                        out=HIT3[:, x : x + SUB, :], in_=HI[:, x : x + SUB, :]
                    )

            phase_a_group0()
            # diag_i = ||hi_i||^2 exactly as the matmul computes it (same bf16
            # inputs, fp32 accumulation).  Emitted right after group 0 so the
            # ScalarE Phase B pipeline is never gated on late DVE work.
            for mi in range(MI):
                jd = scratch.tile([P, P], f32, tag="ttr_junk")
                nc.vector.scalar_tensor_tensor(
                    out=jd,
                    in0=HI[:, mi, :],
                    scalar=1.0,
                    in1=HI[:, mi, :],
                    op0=OP.mult,
                    op1=OP.mult,
                    accum_out=DIAG[:, mi : mi + 1],
                )
            phase_a_group(1)
            phase_a_group(2)
            # positive-pair dots: row-chunk mi pairs tile mi with tile 32+mi
            for mi in range(MI):
                jp = scratch.tile([P, P], f32, tag="ttr_junk")
                nc.vector.scalar_tensor_tensor(
                    out=jp,
                    in0=HI[:, mi, :],
                    scalar=1.0,
                    in1=HI[:, NTILE // 2 + mi, :],
                    op0=OP.mult,
                    op1=OP.mult,
                    accum_out=POS[:, mi : mi + 1],
                )
            phase_a_group(3)

            # ---------------- Phase B: sim row-blocks + exp row-sums --------
            for q in range(GQ):
                for mi in range(MI):
                    lhsT = HIT[:, mi * P : (mi + 1) * P]
                    pg = psum.tile([P, GW], f32, tag="pg")
                    for k in range(GW // 512):
                        nc.tensor.matmul(
                            pg[:, k * 512 : (k + 1) * 512],
                            lhsT,
                            HIT[:, q * GW + k * 512 : q * GW + (k + 1) * 512],
                            start=True,
                            stop=True,
                        )
                    # exp(10*sim - 10) in place on PSUM; accum_out = row sum
                    nc.scalar.activation(
                        out=pg,
                        in_=pg,
                        func=AF.Exp,
                        scale=INV_T,
                        bias=bias_shift,
                        accum_out=SPART[:, mi, q : q + 1],
                    )

            # ---------------- tail: per-row loss -----------------------------
            STOT = persist.tile([P, MI], f32)
            DEXP = persist.tile([P, MI], f32)
            SSUB = persist.tile([P, MI], f32)
            LNS = persist.tile([P, MI], f32)
            OUTA = persist.tile([P, MI], f32)
            OUTF = persist.tile([P, MI], f32)

            nc.vector.reduce_sum(out=STOT, in_=SPART, axis=AX.X)
            nc.scalar.activation(
                out=DEXP, in_=DIAG, func=AF.Exp, scale=INV_T, bias=bias_shift
            )
            nc.vector.tensor_sub(SSUB, STOT, DEXP)
            nc.scalar.activation(out=LNS, in_=SSUB, func=AF.Ln)
            # loss = ln(sum) + SHIFT - INV_T * pos
            nc.vector.scalar_tensor_tensor(
                out=OUTA,
                in0=POS,
                scalar=-INV_T,
                in1=LNS,
                op0=OP.mult,
                op1=OP.add,
            )
            nc.vector.tensor_scalar_add(out=OUTF, in0=OUTA, scalar1=SHIFT)
            nc.sync.dma_start(out=out_h[:, :], in_=OUTF)

    nc.compile()
    return nc


def get_nc():
    if "nc" not in _CACHE:
        _CACHE["nc"] = build_nc()
    return _CACHE["nc"]


def make_in_maps(zis: np.ndarray, zjs: np.ndarray):
    import ml_dtypes

    # representations in reference order: [zjs; zis]
    reps = np.concatenate(
        [np.asarray(zjs, np.float32), np.asarray(zis, np.float32)], axis=0
    )
    maps = []
    for c in range(NCORES):
        rot = np.roll(reps, -ROWS_PER_CORE * c, axis=0)
        tiled = np.ascontiguousarray(
            rot.reshape(NTILE, P, D).transpose(1, 0, 2).reshape(P, TWO_B)
        ).astype(ml_dtypes.bfloat16)
        maps.append({"reps": tiled})
    return maps


def kernel(zis: np.ndarray, zjs: np.ndarray) -> np.ndarray:
    from concourse.bass_utils import run_bass_kernel_spmd

    nc = get_nc()
    in_maps = make_in_maps(zis, zjs)
    res = None
    for attempt in range(3):
        try:
            res = run_bass_kernel_spmd(nc, in_maps, core_ids=list(range(NCORES)))
            break
        except Exception:
            # transient device-unrecoverable states heal on re-execution
            if attempt == 2:
                raise
            import time as _time

            _time.sleep(5.0)
    total = 0.0
    for r in res.results:
        total += float(r["out"].astype(np.float64).sum())
    return np.array(total / TWO_B, dtype=np.float32)



# revision 19
# speedup vs baseline: 1.4215x; 1.4215x over previous
"""NT-Xent loss kernel for Trainium2, 8 NeuronCores.

Problem: B=4096 per view, D=128, temperature=0.1.
reps = concat([zjs, zis]) -> [8192, 128]; normalize rows; sim = normed @ normed.T;
loss = mean_i(-pos_i/T + logsumexp_{j!=i}(sim_ij/T)).

Strategy (fully static SPMD, no collectives) — exploits sim symmetry to
halve the exp work vs a full row-block scan:
  The 8192 rows form 64 tiles of 128.  Row tile T computes only the
  column band [T, T+32] (33 tiles, contiguous in the per-core rotated
  frame): the diagonal tile contributes row sums only; tiles T+1..T+31
  contribute row sums AND column sums (the transposed half of each
  pair); tile T+32 contributes both at host weight 0.5 (pairs at tile
  distance 32 are computed from both sides).  Every unordered pair then
  lands in r_i / r_j exactly once, so the exp covers the full matrix at
  half the cost.  Per core: its 8 row tiles (1024 rows); the rotated
  input keeps the band contiguous, so only 44 of 64 column tiles are
  touched.

  Host prep (like the rotation/tiling/bf16 packing, O(N*D) work that is
  0.01% of the O(N^2*D) kernel): normalize rows in f32 and ship the
  bf16 matmul operand directly in transposed [128d x rows] layout, plus
  a small row-major copy of the 16 tiles needed for pos/diag dots.

  Device, per (row tile, strip in {1536,1536,1152} of the 4224 band):
    PE    sim matmuls -> PSUM  (stationary = row tile, moving = band)
    ACT   exp(10x-10) PSUM -> SBUF bf16 E   (pure exp stream — the
          bottleneck engine runs back-to-back activations)
    DVE   tensor_scalar accumulate row sums of E (+ separate tail sum)
    PE    per-128-col-tile matmul, E as stationary and a ones column as
          moving -> column sums [128,1] into a persistent PSUM
          accumulator (partition-dense, cheap to drain)
  pos_i / diag_i via DVE dot products; diag uses the same bf16 values
  the matmul sees, so the host's exp(10*diag-10) subtraction removes
  the self column exactly.  A dozen warm-up matmuls run during the load
  phase so the PE p-state ramps before the strip pipeline starts.
  Host combines row/col partials (0.5 weight on the distance-32 tail),
  subtracts exp(10 diag - 10), takes log and averages in f64.
"""

import numpy as np

B = 4096
D = 128
TWO_B = 2 * B
P = 128
NCORES = 8
ROWS_PER_CORE = TWO_B // NCORES  # 1024
MI = 8                    # row tiles per core (128 rows each)
NTILES_IN = 44            # band cols reach local tile 40; pad to 44
NSLICES = NTILES_IN // 4
STRIPS = ((0, 1536), (1536, 1536), (3072, 1152))
INV_T = 10.0              # 1 / temperature
SHIFT = 10.0              # fixed logsumexp shift (sim/T <= 10)
OUT_W = 48 + MI * 32      # rowsum/tail/pos/diag block + colsum block

_CACHE = {}


def build_nc():
    import concourse.bacc as bacc
    import concourse.bass as bass
    import concourse.mybir as mybir
    import concourse.tile as tile

    f32 = mybir.dt.float32
    bf16 = mybir.dt.bfloat16
    OP = mybir.AluOpType
    AF = mybir.ActivationFunctionType

    # Pin the act-table chooser to the one set that holds Exp so no
    # mid-kernel ACT_TABLE_LOADs are emitted.
    from concourse import hw_specs

    _orig_tables = hw_specs.get_activation_tables

    def _patched_tables(arch):
        t = {k: set(v) for k, v in _orig_tables(arch).items()}
        for name, s in t.items():
            if name != "natural_log_exp_and_others":
                s.discard(AF.Exp)
                s.discard(AF.Ln)
        return t

    bacc.get_activation_tables = _patched_tables

    nc = bacc.Bacc(
        "TRN2",
        target_bir_lowering=False,
        debug=False,
        num_devices=NCORES,
    )
    # hit[d, 128t+p] = bf16(normed_rot[128t+p, d])  (transposed layout)
    hit_h = nc.declare_dram_parameter("hit", [P, NTILES_IN * P], bf16,
                                      isOutput=False)
    # hirows[p, k*128+d] = bf16(normed_rot[128*T_k+p, d]), T_k: 0..7,32..39
    hir_h = nc.declare_dram_parameter("hirows", [P, 16 * P], bf16,
                                      isOutput=False)
    out_h = nc.declare_dram_parameter("out", [P, OUT_W], f32, isOutput=True)

    with tile.TileContext(nc) as tc:
        with (
            tc.tile_pool(name="persist", bufs=1) as persist,
            tc.tile_pool(name="psum", bufs=2, space="PSUM") as psum,
            tc.tile_pool(name="psumacc", bufs=1, space="PSUM") as psumacc,
            tc.tile_pool(name="escr", bufs=4) as escr,
        ):
            HIT = persist.tile([P, NTILES_IN * P], bf16)
            HIR = persist.tile([P, 16, P], bf16)
            OUTBUF = persist.tile([P, 48], f32)
            ONES = persist.tile([P, 1], bf16)
            JP = persist.tile([P, P], bf16)
            JB = persist.tile([P, 1536], bf16)
            JT = persist.tile([P, P], bf16)
            bias_shift = persist.tile([P, 1], f32)
            nc.vector.memset(ONES, 1.0)
            nc.vector.memset(bias_shift, -SHIFT)

            COLACC = psumacc.tile([P, MI, 32], f32)
            PREHEAT = psumacc.tile([P, 512], f32)

            # ---------------- loads + PE warm-up ----------------------------
            for s in range(NSLICES):
                x, y = 4 * s * P, (4 * s + 4) * P
                nc.gpsimd.dma_start(out=HIT[:, x:y], in_=hit_h[:, x:y])
            nc.scalar.dma_start(
                out=HIR,
                in_=hir_h[:, :].rearrange("p (k d) -> p k d", d=P),
            )
            for _ in range(12):
                nc.tensor.matmul(
                    PREHEAT, HIT[:, 0:P], HIT[:, 0:512],
                    start=True, stop=True,
                )

            def dots(step):
                # diag_i = ||h_i||^2 exactly as the matmul computes it (same
                # bf16 inputs, fp32 accumulation); pos_i = h_i . h_{i+4096}
                for mi in range(MI):
                    k2 = mi if step == 0 else 8 + mi
                    slot = 40 + mi if step == 0 else 32 + mi
                    nc.vector.scalar_tensor_tensor(
                        out=JP, in0=HIR[:, mi, :], scalar=1.0,
                        in1=HIR[:, k2, :], op0=OP.mult, op1=OP.mult,
                        accum_out=OUTBUF[:, slot : slot + 1],
                    )

            # ---------------- strips: sim + exp + row/col sums --------------
            # Strip-major (all A, then B, then C) so early strips only need
            # early HIT slices.  Colsum matmuls queue one strip behind the
            # sims; row sums all on DVE so ACT is a pure exp stream.
            pending_cs = []

            def flush_colsums():
                for lhs, t, m in pending_cs:
                    nc.tensor.matmul(
                        COLACC[:, t, m : m + 1], lhs, ONES,
                        start=True, stop=True,
                    )
                pending_cs.clear()

            for si, (off, w) in enumerate(STRIPS):
                for t in range(MI):
                    base = P * t
                    pg = psum.tile([P, 1536], f32, tag="pg")
                    for k in range(0, w, 512):
                        kw = min(512, w - k)
                        nc.tensor.matmul(
                            pg[:, k : k + kw],
                            HIT[:, base : base + P],
                            HIT[:, base + off + k : base + off + k + kw],
                            start=True, stop=True,
                        )
                    flush_colsums()
                    E = escr.tile([P, 1536], bf16, tag="e")
                    nc.scalar.activation(
                        out=E[:, :w], in_=pg[:, :w], func=AF.Exp,
                        scale=INV_T, bias=bias_shift,
                    )
                    nc.vector.tensor_scalar(
                        out=JB[:, :w], in0=E[:, :w], scalar1=1.0,
                        scalar2=0.0, op0=OP.mult, op1=OP.add,
                        accum_out=OUTBUF[:, 3 * t + si : 3 * t + si + 1],
                    )
                    if si == 2:
                        # tail rowsum for the host's 0.5 correction
                        nc.vector.tensor_scalar(
                            out=JT, in0=E[:, 1024:1152], scalar1=1.0,
                            scalar2=0.0, op0=OP.mult, op1=OP.add,
                            accum_out=OUTBUF[:, 24 + t : 25 + t],
                        )
                    # column sums: E tile stationary, ones moving -> [128,1]
                    j0 = 1 if si == 0 else 0
                    mbase = (0, 11, 23)[si]
                    for j in range(j0, w // P):
                        pending_cs.append(
                            (E[:, j * P : (j + 1) * P], t, mbase + j - j0)
                        )
                    if si == 0 and t == 1:
                        dots(0)
                    elif si == 0 and t == 2:
                        dots(1)
            flush_colsums()

            CSOUT = persist.tile([P, MI * 32], f32)
            nc.vector.tensor_scalar(
                out=CSOUT, in0=COLACC.rearrange("p t m -> p (t m)"),
                scalar1=1.0, scalar2=None, op0=OP.mult,
            )
            nc.sync.dma_start(out=out_h[:, 0:48], in_=OUTBUF)
            nc.sync.dma_start(out=out_h[:, 48:OUT_W], in_=CSOUT)

    nc.compile()
    return nc


def get_nc():
    if "nc" not in _CACHE:
        _CACHE["nc"] = build_nc()
    return _CACHE["nc"]


def make_in_maps(zis: np.ndarray, zjs: np.ndarray):
    import ml_dtypes

    # representations in reference order: [zjs; zis], normalized rows
    # (f32 norms with the torch CosineSimilarity 1e-8 clamp)
    reps = np.concatenate(
        [np.asarray(zjs, np.float32), np.asarray(zis, np.float32)], axis=0
    )
    normed = (
        reps / np.maximum(np.linalg.norm(reps, axis=1, keepdims=True), 1e-8)
    ).astype(ml_dtypes.bfloat16)
    maps = []
    kt = np.r_[0:8, 32:40]
    for c in range(NCORES):
        rot = np.roll(normed, -ROWS_PER_CORE * c, axis=0)[: NTILES_IN * P]
        hit = np.ascontiguousarray(rot.T)            # [128 d, 5632 rows]
        hir = np.ascontiguousarray(
            rot.reshape(NTILES_IN, P, D)[kt].transpose(1, 0, 2).reshape(
                P, 16 * P
            )
        )
        maps.append({"hit": hit, "hirows": hir})
    return maps


def kernel(zis: np.ndarray, zjs: np.ndarray) -> np.ndarray:
    from concourse.bass_utils import run_bass_kernel_spmd

    nc = get_nc()
    in_maps = make_in_maps(zis, zjs)
    res = None
    for attempt in range(3):
        try:
            res = run_bass_kernel_spmd(nc, in_maps, core_ids=list(range(NCORES)))
            break
        except Exception:
            # transient device-unrecoverable states heal on re-execution
            if attempt == 2:
                raise
            import time as _time

            _time.sleep(5.0)

    # ---- host combine (f64) -------------------------------------------
    r = np.zeros(TWO_B, dtype=np.float64)
    pos = np.zeros(TWO_B, dtype=np.float64)
    diag = np.zeros(TWO_B, dtype=np.float64)

    p_idx = np.arange(P)
    t_idx = np.arange(MI)
    m_idx = np.arange(32)
    row_l = 128 * t_idx[None, :] + p_idx[:, None]              # [P, MI]
    col_l = (128 * (t_idx[None, :, None] + 1 + m_idx[None, None, :])
             + p_idx[:, None, None])                           # [P, MI, 32]
    cw = np.where(m_idx == 31, 0.5, 1.0)[None, None, :]

    for c, rr in enumerate(res.results):
        o = rr["out"].astype(np.float64)                       # [P, OUT_W]
        rsum = o[:, 0:24].reshape(P, MI, 3)
        tail = o[:, 24:32]
        csum = o[:, 48:OUT_W].reshape(P, MI, 32)
        g_row = (1024 * c + row_l) % TWO_B
        g_col = (1024 * c + col_l) % TWO_B
        np.add.at(r, g_row,
                  rsum[:, :, 0] + rsum[:, :, 1] + rsum[:, :, 2]
                  - 0.5 * tail)
        np.add.at(r, g_col, cw * csum)
        pos[g_row] = o[:, 32:40]
        diag[g_row] = o[:, 40:48]

    lse = np.log(r - np.exp(INV_T * diag - SHIFT)) + SHIFT
    loss = np.mean(-INV_T * pos + lse)
    return np.array(loss, dtype=np.float32)
